# revision 14
# baseline (speedup 1.0000x reference)
"""DistanceFromAnswerLoss on 8 Trainium2 NeuronCores.

out = 0.1 * sum_{b,c} mask[b,c] * exp(input[b,c])
  mask[b,c] = |c - t_b| / sqrt(sum_c (c - t_b)^2),  mask = 0 where t_b == 0

Sharding: data-parallel over the batch dim (512 rows per core); each core
computes its partial 0.1 * sum, host adds the 8 scalars.

Per-core per-element pipeline (memory-bound; one pass per engine):
  ScalarE : e = exp(x)
  GpSimd  : d = |iota - t|           (tensor_scalar subtract + abs_max 0)
  VectorE : row_acc += sum(d * e)    (tensor_tensor_reduce, fused mul+reduce)
The row norm uses the closed form  sum_c (c-t)^2 = C*(t-mu)^2 + K  so no
second pass over the data is needed.  Final 128-partition reduction is a
[128,1]x[128,RB] matmul against a ones vector on the otherwise-idle PE.
"""

import sys
from contextlib import ExitStack

import numpy as np

sys.path.insert(0, "/opt/trn_rl_repo")

import concourse.bass as bass
import concourse.tile as tile
from concourse import bacc, mybir
from concourse.bass_utils import run_bass_kernel_spmd

B = 4096
C = 8192
N_CORES = 8
ROWS = B // N_CORES          # 512 rows per core
RB = ROWS // 128             # 4 row blocks of 128 partitions
W = 4096                     # column tile width (2 MiB DMAs)
NW = C // W
COEFF = 0.1

MU = (C - 1) / 2.0
_S1 = (C - 1) * C // 2
_S2 = (C - 1) * C * (2 * C - 1) // 6
K = float(_S2 - _S1 * _S1 / C)   # sum_c (c-t)^2 = C*(t-MU)^2 + K

F32 = mybir.dt.float32
BF16 = mybir.dt.bfloat16


def _build() -> bass.Bass:
    nc = bacc.Bacc("TRN2", target_bir_lowering=False, debug=False)
    x = nc.declare_dram_parameter("x", [RB, 128, C], F32, isOutput=False)
    t = nc.declare_dram_parameter("t", [RB, 128, 1], F32, isOutput=False)
    out = nc.declare_dram_parameter("out", [1, 1], F32, isOutput=True)

    with tile.TileContext(nc) as tc, ExitStack() as ctx:
        const_pool = ctx.enter_context(tc.tile_pool(name="const", bufs=1))
        xpool = ctx.enter_context(tc.tile_pool(name="x", bufs=4))
        epool = ctx.enter_context(tc.tile_pool(name="e", bufs=2))
        dpool = ctx.enter_context(tc.tile_pool(name="d", bufs=2))
        jpool = ctx.enter_context(tc.tile_pool(name="j", bufs=2))
        spool = ctx.enter_context(tc.tile_pool(name="s", bufs=2))
        psum_pool = ctx.enter_context(tc.tile_pool(name="ps", bufs=1, space="PSUM"))

        # bf16 iota: c values round to multiples of 2^(ceil(log2 c)-8) above
        # 256, so dist is off by <=16 on average — washes out over the 33.5M
        # positive summands (measured 4e-5 total rel err) and buys the DVE
        # 4x packed mode for the dist pass.
        iota = const_pool.tile([128, C], BF16)
        nc.gpsimd.iota(
            iota[:],
            pattern=[[1, C]],
            base=0,
            channel_multiplier=0,
            allow_small_or_imprecise_dtypes=True,
        )
        ones = const_pool.tile([128, 1], F32)
        nc.vector.memset(ones[:], 1.0)
        negmu = const_pool.tile([128, 1], F32)
        nc.vector.memset(negmu[:], -MU)
        partials = const_pool.tile([128, RB], F32)

        for rb in range(RB):
            tcol = spool.tile([128, 1], F32)
            nc.sync.dma_start(out=tcol[:], in_=t[rb])

            # norm^2 = C*(t-MU)^2 + K ; scale = 0.1 * (t != 0) / norm
            tsq = spool.tile([128, 1], F32)
            nc.scalar.activation(
                tsq[:], tcol[:], mybir.ActivationFunctionType.Square, bias=negmu[:]
            )
            n2 = spool.tile([128, 1], F32)
            nc.vector.tensor_scalar(
                n2[:], tsq[:], float(C), K,
                op0=mybir.AluOpType.mult, op1=mybir.AluOpType.add,
            )
            norm = spool.tile([128, 1], F32)
            nc.scalar.activation(norm[:], n2[:], mybir.ActivationFunctionType.Sqrt)
            inv = spool.tile([128, 1], F32)
            nc.vector.reciprocal(inv[:], norm[:])
            nz = spool.tile([128, 1], F32)
            nc.vector.tensor_scalar(
                nz[:], tcol[:], 0.0, None, op0=mybir.AluOpType.not_equal
            )
            scale = spool.tile([128, 1], F32)
            nc.vector.tensor_scalar(
                scale[:], inv[:], nz[:], COEFF,
                op0=mybir.AluOpType.mult, op1=mybir.AluOpType.mult,
            )

            # |c-t|*e = max(c,t)*e - min(c,t)*e : two fused scalar_tensor_tensor
            # ops (2x bf16 DVE mode) with built-in row accumulation — no
            # separate dist tile, no abs.
            accM = spool.tile([128, NW], F32)
            accN = spool.tile([128, NW], F32)
            for cw in range(NW):
                xt = xpool.tile([128, W], F32)
                nc.sync.dma_start(out=xt[:], in_=x[rb, :, cw * W:(cw + 1) * W])
                et = epool.tile([128, W], BF16)
                nc.scalar.activation(et[:], xt[:], mybir.ActivationFunctionType.Exp)
                jmax = dpool.tile([128, W], BF16)
                nc.vector.scalar_tensor_tensor(
                    out=jmax[:], in0=iota[:, cw * W:(cw + 1) * W], scalar=tcol[:],
                    in1=et[:], op0=mybir.AluOpType.max, op1=mybir.AluOpType.mult,
                    accum_out=accM[:, cw:cw + 1],
                )
                jmin = jpool.tile([128, W], BF16)
                nc.vector.scalar_tensor_tensor(
                    out=jmin[:], in0=iota[:, cw * W:(cw + 1) * W], scalar=tcol[:],
                    in1=et[:], op0=mybir.AluOpType.min, op1=mybir.AluOpType.mult,
                    accum_out=accN[:, cw:cw + 1],
                )

            diff = spool.tile([128, NW], F32)
            nc.vector.tensor_sub(diff[:], accM[:], accN[:])
            rowacc = spool.tile([128, 1], F32)
            nc.vector.tensor_reduce(
                rowacc[:], diff[:], axis=mybir.AxisListType.X, op=mybir.AluOpType.add
            )
            nc.vector.tensor_scalar(
                partials[:, rb:rb + 1], rowacc[:], scale[:], None,
                op0=mybir.AluOpType.mult,
            )

        # cross-partition reduce: [1,RB] = ones[128,1].T @ partials[128,RB]
        ptot = psum_pool.tile([1, RB], F32)
        nc.tensor.matmul(ptot[:], ones[:], partials[:], start=True, stop=True)
        tot = const_pool.tile([1, 1], F32)
        nc.vector.tensor_reduce(
            tot[:], ptot[:], axis=mybir.AxisListType.X, op=mybir.AluOpType.add
        )
        nc.sync.dma_start(out=out[:, :], in_=tot[:])

    nc.finalize()
    return nc


_NC = None


def _get_nc() -> bass.Bass:
    global _NC
    if _NC is None:
        _NC = _build()
    return _NC


def make_in_maps(input: np.ndarray, target: np.ndarray) -> list[dict]:
    x = np.ascontiguousarray(np.asarray(input, dtype=np.float32)).reshape(
        N_CORES, RB, 128, C
    )
    t = np.asarray(target).astype(np.float32).reshape(N_CORES, RB, 128, 1)
    return [{"x": x[i], "t": t[i]} for i in range(N_CORES)]


def run(input: np.ndarray, target: np.ndarray, trace: bool = False, tmpdir=None):
    nc = _get_nc()
    in_maps = make_in_maps(input, target)
    res = run_bass_kernel_spmd(
        nc, in_maps, list(range(N_CORES)), trace=trace, tmpdir=tmpdir
    )
    total = np.float32(0.0)
    for r in res.results:
        total += np.float32(r["out"].reshape(-1)[0])
    return np.asarray(total, dtype=np.float32), res


def kernel(input: np.ndarray, target: np.ndarray) -> np.ndarray:
    out, _ = run(input, target)
    return out


# revision 18
# speedup vs baseline: 1.0588x; 1.0588x over previous
"""DistanceFromAnswerLoss on 8 Trainium2 NeuronCores.

out = 0.1 * sum_{b,c} mask[b,c] * exp(input[b,c])
  mask[b,c] = |c - t_b| / sqrt(sum_c (c - t_b)^2),  mask = 0 where t_b == 0

Sharding: data-parallel over the batch dim (512 rows per core); each core
computes its partial 0.1 * sum, host adds the 8 scalars.

Per-core pipeline (memory-bound target: ~47us DMA floor at 360 GB/s):
  ScalarE : e = exp(x) -> bf16 (one pass, 1.2 GHz, table set exp_and_others)
  VectorE : d = iota - t        (tensor_scalar ptr, 2x/4x packed bf16 mode)
            p = d * e           (tensor_tensor, 2x bf16 mode)
  abs+row-reduce, split to balance engines:
    K_ACT tiles: ScalarE activation(Abs, accum_out) — same table set as exp,
                 so no table reloads; gives sum|p| per partition for free.
    rest:        two scalar_tensor_tensor ops (1x) using
                 |c-t|*e = max(c,t)*e - min(c,t)*e.
The row norm uses the closed form sum_c (c-t)^2 = C*(t-mu)^2 + K computed
once for all 512 rows on [128,4] tiles.  Final 128-partition reduction is a
tiny matmul against a ones vector on the otherwise-idle PE.
"""

import sys
from contextlib import ExitStack

import numpy as np

sys.path.insert(0, "/opt/trn_rl_repo")

import concourse.bass as bass
import concourse.tile as tile
from concourse import bacc, mybir
from concourse.bass_utils import run_bass_kernel_spmd

B = 4096
C = 8192
N_CORES = 8
ROWS = B // N_CORES          # 512 rows per core
RB = ROWS // 128             # 4 row blocks of 128 partitions
W = 4096                     # column tile width (2 MiB DMAs)
NW = C // W
NT = RB * NW                 # 8 big tiles per core
K_ACT = 5                    # tiles whose abs+reduce runs on ScalarE
COEFF = 0.1

MU = (C - 1) / 2.0
_S1 = (C - 1) * C // 2
_S2 = (C - 1) * C * (2 * C - 1) // 6
K = float(_S2 - _S1 * _S1 / C)   # sum_c (c-t)^2 = C*(t-MU)^2 + K

F32 = mybir.dt.float32
BF16 = mybir.dt.bfloat16
Af = mybir.ActivationFunctionType
Op = mybir.AluOpType


def _build() -> bass.Bass:
    nc = bacc.Bacc("TRN2", target_bir_lowering=False, debug=False)
    x = nc.declare_dram_parameter("x", [RB, 128, C], F32, isOutput=False)
    # t arrives partition-major: t_host[p, rb] = target[rb*128 + p]
    t = nc.declare_dram_parameter("t", [128, RB], F32, isOutput=False)
    out = nc.declare_dram_parameter("out", [1, 1], F32, isOutput=True)

    with tile.TileContext(nc) as tc, ExitStack() as ctx:
        const_pool = ctx.enter_context(tc.tile_pool(name="const", bufs=1))
        xpool = ctx.enter_context(tc.tile_pool(name="x", bufs=3))
        epool = ctx.enter_context(tc.tile_pool(name="e", bufs=2))
        dpool = ctx.enter_context(tc.tile_pool(name="d", bufs=2))
        ppool = ctx.enter_context(tc.tile_pool(name="p", bufs=2))
        jpool = ctx.enter_context(tc.tile_pool(name="j", bufs=3))
        spool = ctx.enter_context(tc.tile_pool(name="s", bufs=1))
        psum_pool = ctx.enter_context(tc.tile_pool(name="ps", bufs=1, space="PSUM"))

        # --- tiny front matter: t block + norm chain on [128, RB] ---------
        ttile = const_pool.tile([128, RB], F32)
        nc.sync.dma_start(out=ttile[:], in_=t[:, :])
        negmu = const_pool.tile([128, 1], F32)
        nc.vector.memset(negmu[:], -MU)
        ones = const_pool.tile([128, 1], F32)
        nc.vector.memset(ones[:], 1.0)

        # fold COEFF into the norm: 1/sqrt(100*n2) = 0.1/sqrt(n2)
        tsq = spool.tile([128, RB], F32)
        nc.scalar.activation(tsq[:], ttile[:], Af.Square, bias=negmu[:])
        n2 = spool.tile([128, RB], F32)
        nc.vector.tensor_scalar(
            n2[:], tsq[:], float(C) / COEFF**2, K / COEFF**2,
            op0=Op.mult, op1=Op.add,
        )
        norm = spool.tile([128, RB], F32)
        nc.scalar.activation(norm[:], n2[:], Af.Sqrt)
        inv = spool.tile([128, RB], F32)
        nc.vector.reciprocal(inv[:], norm[:])
        nz = spool.tile([128, RB], F32)
        nc.vector.tensor_scalar(nz[:], ttile[:], 0.0, None, op0=Op.not_equal)
        scale = spool.tile([128, RB], F32)
        nc.vector.tensor_tensor(scale[:], inv[:], nz[:], op=Op.mult)

        # --- constants: bf16 iota in W/2-chunks so tile 0 unblocks early --
        iota = const_pool.tile([128, C], BF16)
        CH = 2048
        for ci in range(C // CH):
            nc.gpsimd.iota(
                iota[:, ci * CH:(ci + 1) * CH],
                pattern=[[1, CH]],
                base=ci * CH,
                channel_multiplier=0,
                allow_small_or_imprecise_dtypes=True,
            )

        # --- main loop over the 8 [128, W] tiles --------------------------
        acc_all = const_pool.tile([128, NT], F32)
        for idx in range(NT):
            rb, cw = divmod(idx, NW)
            tcol = ttile[:, rb:rb + 1]
            xt = xpool.tile([128, W], F32)
            nc.sync.dma_start(out=xt[:], in_=x[rb, :, cw * W:(cw + 1) * W])
            et = epool.tile([128, W], BF16)
            nc.scalar.activation(et[:], xt[:], Af.Exp)
            if idx < K_ACT:
                dt = dpool.tile([128, W], BF16)
                nc.vector.tensor_scalar(
                    dt[:], iota[:, cw * W:(cw + 1) * W], tcol, None,
                    op0=Op.subtract,
                )
                pt = ppool.tile([128, W], BF16)
                nc.vector.tensor_tensor(pt[:], dt[:], et[:], op=Op.mult)
                jt = jpool.tile([128, W], BF16)
                nc.scalar.activation(
                    jt[:], pt[:], Af.Abs, accum_out=acc_all[:, idx:idx + 1]
                )
            else:
                jmax = jpool.tile([128, W], BF16)
                aM = spool.tile([128, 1], F32, tag=f"am{idx}")
                nc.vector.scalar_tensor_tensor(
                    out=jmax[:], in0=iota[:, cw * W:(cw + 1) * W], scalar=tcol,
                    in1=et[:], op0=Op.max, op1=Op.mult, accum_out=aM[:],
                )
                jmin = jpool.tile([128, W], BF16)
                aN = spool.tile([128, 1], F32, tag=f"an{idx}")
                nc.vector.scalar_tensor_tensor(
                    out=jmin[:], in0=iota[:, cw * W:(cw + 1) * W], scalar=tcol,
                    in1=et[:], op0=Op.min, op1=Op.mult, accum_out=aN[:],
                )
                nc.vector.tensor_sub(acc_all[:, idx:idx + 1], aM[:], aN[:])

        # --- combine: rowacc[128, RB] -> scaled -> cross-partition sum ----
        rowacc = spool.tile([128, RB], F32)
        nc.vector.tensor_reduce(
            rowacc[:], acc_all[:].rearrange("p (rb nw) -> p rb nw", nw=NW),
            axis=mybir.AxisListType.X, op=Op.add,
        )
        partials = spool.tile([128, RB], F32)
        nc.vector.tensor_tensor(partials[:], rowacc[:], scale[:], op=Op.mult)
        ptot = psum_pool.tile([1, RB], F32)
        nc.tensor.matmul(ptot[:], ones[:], partials[:], start=True, stop=True)
        tot = spool.tile([1, 1], F32)
        nc.vector.tensor_reduce(
            tot[:], ptot[:], axis=mybir.AxisListType.X, op=Op.add
        )
        nc.sync.dma_start(out=out[:, :], in_=tot[:])

    nc.finalize()
    return nc


_NC = None


def _get_nc() -> bass.Bass:
    global _NC
    if _NC is None:
        _NC = _build()
    return _NC


def make_in_maps(input: np.ndarray, target: np.ndarray) -> list[dict]:
    x = np.ascontiguousarray(np.asarray(input, dtype=np.float32)).reshape(
        N_CORES, RB, 128, C
    )
    # [N_CORES, 128, RB] partition-major targets
    t = np.asarray(target).astype(np.float32).reshape(N_CORES, RB, 128)
    t = np.ascontiguousarray(t.transpose(0, 2, 1))
    return [{"x": x[i], "t": t[i]} for i in range(N_CORES)]


def run(input: np.ndarray, target: np.ndarray, trace: bool = False, tmpdir=None):
    nc = _get_nc()
    in_maps = make_in_maps(input, target)
    res = run_bass_kernel_spmd(
        nc, in_maps, list(range(N_CORES)), trace=trace, tmpdir=tmpdir
    )
    total = np.float32(0.0)
    for r in res.results:
        total += np.float32(r["out"].reshape(-1)[0])
    return np.asarray(total, dtype=np.float32), res


def kernel(input: np.ndarray, target: np.ndarray) -> np.ndarray:
    out, _ = run(input, target)
    return out
